# revision 19
# baseline (speedup 1.0000x reference)
"""Bass/Tile SPMD kernel for GQA attention prefill (B=2,S=2048,D=4096,H=32,KVH=8,HD=128).

Head-sharded layout: 8 cores = 2 batch-groups x 4 head-group cores.
Core c: batch b=c//4, head-group hg=c%4 owns q-heads [8hg, 8hg+8) and
kv-heads [2hg, 2hg+2), projecting them for ALL 2048 rows of its batch.
No K/V/Q collectives: every core computes exactly the Q/K/V it needs.

Per core:
  pass A: K_T + V projections (2 kv-heads, all 2048 rows), x streamed in
          4 column-chunks of 512; RoPE on K.
  pass B: Q_T projection + RoPE (8 heads, all rows), wq resident, x
          re-streamed.
  phase 2: exact-causal attention per head over 4 query-slices of 512
          (blocks 2i,2i+1). k-tiles 0..4i+1 run 512-wide; the two tail
          k-tiles (diagonal of block 2i+1) run 256-wide. exp on ScalarE
          over [128,1024] 2-bank PSUM pairs; 0/1 mask only on diagonal
          tiles (one shared [128,2,256] tile - relative positions are
          block-invariant); rowsum via ones-matmul; normalize via
          reciprocal_approx_fast + PE broadcast-matmul.
  a2a:    attention outputs AllToAll'd inside the batch group so core c
          ends with all 32 heads for its 512 query rows.
  phase 3: out rows = attn_T @ wo with full wo streamed once in 8 big
          chunks. Output rows disjoint across cores -> no reduction.
"""
import numpy as np
import concourse.bass as bass
import concourse.mybir as mybir
import concourse.tile as tile
from concourse import bacc

F32 = mybir.dt.float32
BF16 = mybir.dt.bfloat16

B, S, D = 2, 2048, 4096
H, KVH, HD = 32, 8, 128
NDT = D // 128              # 32 contraction tiles
NQH, NKVH = 8, 2            # heads per core
NCH, CW = 4, 512            # x column chunks
SCALE = float(1.0 / np.sqrt(HD))
A2A_CHUNK = NQH * 128 * 512  # elems sent to each peer

REPLICA_GROUPS = [[0, 1, 2, 3], [4, 5, 6, 7]]


def build():
    nc = bacc.Bacc("TRN2", target_bir_lowering=False, debug=False, num_devices=8)

    xt = nc.declare_dram_parameter("xt", [D, S], BF16, isOutput=False)
    wqt = nc.declare_dram_parameter("wqt", [NQH, NDT, 128, 128], BF16, isOutput=False)
    wkt = nc.declare_dram_parameter("wkt", [NDT, 128, NKVH * 128], BF16, isOutput=False)
    wvt = nc.declare_dram_parameter("wvt", [NDT, 128, NKVH * 128], BF16, isOutput=False)
    wot = nc.declare_dram_parameter("wot", [NQH * HD, D], BF16, isOutput=False)
    cos2 = nc.declare_dram_parameter("cos2", [128, S], BF16, isOutput=False)
    sin2 = nc.declare_dram_parameter("sin2", [128, S], BF16, isOutput=False)
    mdiag = nc.declare_dram_parameter("mdiag", [128, 512], BF16, isOutput=False)
    out = nc.declare_dram_parameter("out", [512, D], F32, isOutput=True)

    with tile.TileContext(nc) as tc:
        _body(nc, tc, xt.ap(), wqt.ap(), wkt.ap(), wvt.ap(), wot.ap(),
              cos2.ap(), sin2.ap(), mdiag.ap(), out.ap())

    nc.compile()
    return nc


def _body(nc, tc, xt, wqt, wkt, wvt, wot, cos2, sin2, mdiag, out):
    from contextlib import ExitStack

    es = ExitStack()
    with es:
        const_pool = es.enter_context(tc.tile_pool(name="consts", bufs=1))
        dram = es.enter_context(tc.tile_pool(name="dram", bufs=1, space="DRAM"))
        qkv_pool = es.enter_context(tc.tile_pool(name="qkv", bufs=1))

        cos_sb = const_pool.tile([128, S], BF16)
        sin_sb = const_pool.tile([128, S], BF16)
        md_sb = const_pool.tile([128, 2, 256], BF16)
        ones_sb = const_pool.tile([128, 1], BF16)     # rowsum stationary
        nc.sync.dma_start(cos_sb[:], cos2)
        nc.sync.dma_start(sin_sb[:], sin2)
        nc.sync.dma_start(md_sb[:], mdiag.rearrange("p (h c) -> p h c", c=256))
        nc.vector.memset(ones_sb[:], 1.0)

        q_sb = qkv_pool.tile([128, NQH, S], BF16)     # Q_T head-major
        k_sb = qkv_pool.tile([128, NKVH, S], BF16)    # K_T kv-head-major
        v_sb = qkv_pool.tile([128, S // 128, NKVH * 128], BF16)  # V natural
        # phase-3 ReduceScatter staging: per 512-wide d-chunk, the partial
        # y (all 2048 rows, this core's 8 heads) and the scattered result
        y_dram = [dram.tile([16 * 128 * 512], F32, name=f"ydram{i}")
                  for i in range(8)]
        rs_out = [dram.tile([512 * 512], F32, name=f"rsout{i}")
                  for i in range(8)]

        xtr = xt.rearrange("(dt p) s -> p dt s", p=128)

        # ================= pass A: K/V projections =================
        with tc.tile_pool(name="pAw", bufs=1) as wpool, \
             tc.tile_pool(name="pAx", bufs=2) as xpool, \
             tc.tile_pool(name="pArope", bufs=2) as rpool, \
             tc.tile_pool(name="pAps", bufs=2, space="PSUM") as pspool, \
             tc.tile_pool(name="pApsv", bufs=2, space="PSUM") as psvpool:

            wk_sb = wpool.tile([128, NDT, NKVH * 128], BF16)
            wv_sb = wpool.tile([128, NDT, NKVH * 128], BF16)
            nc.sync.dma_start(wk_sb[:], wkt.rearrange("dt p e -> p dt e"))
            nc.sync.dma_start(wv_sb[:], wvt.rearrange("dt p e -> p dt e"))

            def rope(ps, dst, qc):
                raw = rpool.tile([128, CW], BF16, tag="rope_raw")
                sw_t = rpool.tile([128, CW], BF16, tag="rope_sw")
                nc.vector.tensor_copy(raw[:], ps)
                nc.scalar.dma_start(sw_t[0:64, :], raw[64:128, :])
                nc.scalar.dma_start(sw_t[64:128, :], raw[0:64, :])
                nc.vector.tensor_mul(dst, ps, cos_sb[:, qc * CW:(qc + 1) * CW])
                nc.vector.tensor_mul(
                    sw_t[:], sw_t[:], sin_sb[:, qc * CW:(qc + 1) * CW])
                nc.vector.tensor_add(dst, dst, sw_t[:])

            for qc in range(NCH):
                x = xpool.tile([128, NDT, CW], BF16, tag="xchunk")
                nc.sync.dma_start(x[:], xtr[:, :, qc * CW:(qc + 1) * CW])
                for kv in range(NKVH):
                    kps = pspool.tile([128, CW], F32, tag="kps")
                    for dt in range(NDT):
                        nc.tensor.matmul(
                            kps[:], wk_sb[:, dt, kv * 128:(kv + 1) * 128],
                            x[:, dt], start=(dt == 0), stop=(dt == NDT - 1))
                    rope(kps[:], k_sb[:, kv, qc * CW:(qc + 1) * CW], qc)
                vps = psvpool.tile([128, 4, 256], F32, tag="vps")
                # st outer: one live accumulation group per PSUM bank at a
                # time (a second start=True in the same bank clears the
                # bank's has_written bits and drops the first group's sum)
                for st in range(4):
                    for dt in range(NDT):
                        nc.tensor.matmul(
                            vps[:, st], x[:, dt, st * 128:(st + 1) * 128],
                            wv_sb[:, dt], start=(dt == 0), stop=(dt == NDT - 1))
                    nc.vector.tensor_copy(v_sb[:, qc * 4 + st, :], vps[:, st])

        # ================= pass B: Q projection =================
        with tc.tile_pool(name="pBw", bufs=1) as wqpool, \
             tc.tile_pool(name="pBx", bufs=2) as xpool, \
             tc.tile_pool(name="pBrope", bufs=2) as rpool, \
             tc.tile_pool(name="pBps", bufs=3, space="PSUM") as pspool:

            wq_sb = wqpool.tile([128, NQH, NDT, 128], BF16)
            for et in range(NQH):
                nc.sync.dma_start(
                    wq_sb[:, et], wqt[et].rearrange("dt p e -> p dt e"))

            def ropeq(ps, dst, qc):
                raw = rpool.tile([128, CW], BF16, tag="rope_raw")
                sw_t = rpool.tile([128, CW], BF16, tag="rope_sw")
                nc.vector.tensor_copy(raw[:], ps)
                nc.scalar.dma_start(sw_t[0:64, :], raw[64:128, :])
                nc.scalar.dma_start(sw_t[64:128, :], raw[0:64, :])
                nc.vector.tensor_mul(dst, ps, cos_sb[:, qc * CW:(qc + 1) * CW])
                nc.vector.tensor_mul(
                    sw_t[:], sw_t[:], sin_sb[:, qc * CW:(qc + 1) * CW])
                nc.vector.tensor_add(dst, dst, sw_t[:])

            for qc in range(NCH):
                x = xpool.tile([128, NDT, CW], BF16, tag="xchunk")
                nc.sync.dma_start(x[:], xtr[:, :, qc * CW:(qc + 1) * CW])
                for et in range(NQH):
                    qps = pspool.tile([128, CW], F32, tag="qps")
                    for dt in range(NDT):
                        nc.tensor.matmul(
                            qps[:], wq_sb[:, et, dt], x[:, dt],
                            start=(dt == 0), stop=(dt == NDT - 1))
                    ropeq(qps[:], q_sb[:, et, qc * CW:(qc + 1) * CW], qc)

        # ================= phase 2: exact-causal attention =================
        attn_pool = es.enter_context(tc.tile_pool(name="attn", bufs=1))
        attn_sb = attn_pool.tile([128, NQH, S], BF16)

        with tc.tile_pool(name="p2p", bufs=2) as ppool, \
             tc.tile_pool(name="p2n", bufs=3) as npool, \
             tc.tile_pool(name="p2ps_s", bufs=2, space="PSUM") as ps_s, \
             tc.tile_pool(name="p2ps_o", bufs=2, space="PSUM") as ps_o, \
             tc.tile_pool(name="p2ps_r", bufs=2, space="PSUM") as ps_r:

            for h in range(NQH):
                kv = h // 4
                for i in range(4):          # query slice: blocks 2i, 2i+1
                    q0 = i * 512
                    npair = 2 * i + 1       # 512-wide kt pairs
                    # p_big[p, t, half, q]: exp'd probs, key-tile-pair major
                    p_big = ppool.tile([128, 8, 2, 512], BF16, tag="p_big")
                    ops = ps_o.tile([128, 512], F32, tag="ops")
                    rps = ps_r.tile([1, 512], F32, tag="rps")

                    def qk_pair(t):
                        sps = ps_s.tile([128, 2, 512], F32, tag="sps")
                        nc.tensor.matmul(
                            sps[:, 0], k_sb[:, kv, (2 * t) * 128:(2 * t + 1) * 128],
                            q_sb[:, h, q0:q0 + 512], start=True, stop=True)
                        nc.tensor.matmul(
                            sps[:, 1], k_sb[:, kv, (2 * t + 1) * 128:(2 * t + 2) * 128],
                            q_sb[:, h, q0:q0 + 512], start=True, stop=True)
                        nc.scalar.activation(
                            p_big[:, t], sps[:],
                            mybir.ActivationFunctionType.Exp)
                        if t == npair - 1:
                            # diagonal of block 2i: cols 0:256 of both halves
                            nc.vector.tensor_mul(
                                p_big[:, t, :, 0:256], p_big[:, t, :, 0:256],
                                md_sb[:])

                    def qk_tail():
                        # kt=4i+2, 4i+3: diagonal of block 2i+1 (cols 256:512)
                        tps = ps_s.tile([128, 2, 512], F32, tag="sps")
                        nc.tensor.matmul(
                            tps[:, 0, 0:256],
                            k_sb[:, kv, (4 * i + 2) * 128:(4 * i + 3) * 128],
                            q_sb[:, h, q0 + 256:q0 + 512], start=True, stop=True)
                        nc.tensor.matmul(
                            tps[:, 1, 0:256],
                            k_sb[:, kv, (4 * i + 3) * 128:(4 * i + 4) * 128],
                            q_sb[:, h, q0 + 256:q0 + 512], start=True, stop=True)
                        nc.scalar.activation(
                            p_big[:, npair, :, 0:256], tps[:, :, 0:256],
                            mybir.ActivationFunctionType.Exp)
                        nc.vector.tensor_mul(
                            p_big[:, npair, :, 0:256], p_big[:, npair, :, 0:256],
                            md_sb[:])

                    def pv_pair(t):
                        for half in range(2):
                            nc.tensor.matmul(
                                ops[:],
                                v_sb[:, 2 * t + half, kv * 128:(kv + 1) * 128],
                                p_big[:, t, half], start=(t == 0 and half == 0),
                                stop=False, skip_group_check=True)
                            nc.tensor.matmul(
                                rps[:], ones_sb[:], p_big[:, t, half],
                                start=(t == 0 and half == 0), stop=False,
                                skip_group_check=True)

                    def pv_tail():
                        for half in range(2):
                            nc.tensor.matmul(
                                ops[:, 256:512],
                                v_sb[:, 4 * i + 2 + half, kv * 128:(kv + 1) * 128],
                                p_big[:, npair, half, 0:256],
                                start=False, stop=(half == 1),
                                skip_group_check=True)
                            nc.tensor.matmul(
                                rps[0:1, 256:512], ones_sb[:],
                                p_big[:, npair, half, 0:256],
                                start=False, stop=(half == 1),
                                skip_group_check=True)

                    # software pipeline: QK(t+1) issued before PV(t) so the
                    # exp of segment t overlaps PE work on segment t+1
                    qk_pair(0)
                    for t in range(1, npair):
                        qk_pair(t)
                        pv_pair(t - 1)
                    qk_tail()
                    pv_pair(npair - 1)
                    pv_tail()
                    # --- normalize ---
                    rcp = npool.tile([1, 512], F32, tag="rcp")
                    nc.vector.reciprocal_approx_fast(rcp[:], rps[0:1, :])
                    rcpb = npool.tile([128, 512], F32, tag="rcpb")
                    nc.gpsimd.partition_broadcast(rcpb[:], rcp[:])
                    raw = npool.tile([128, 512], BF16, tag="attn_raw")
                    nc.vector.tensor_copy(raw[:], ops[:])
                    nc.vector.tensor_mul(
                        attn_sb[:, h, q0:q0 + 512], raw[:], rcpb[:])

        # ===== phase 3: head-sharded out-proj + per-chunk ReduceScatter ====
        # y_partial[2048, dch*512:+512] = sum over this core's 8 heads of
        # attn_T^T @ wo_rows; ReduceScatter(add) over the batch group then
        # yields each core's own 512 query rows, already summed over all
        # 32 heads.
        with tc.tile_pool(name="p3w", bufs=1) as wopool, \
             tc.tile_pool(name="p3y", bufs=8) as ypool, \
             tc.tile_pool(name="p3ps", bufs=2, space="PSUM") as ps_y:

            # wo rows for this core's heads: [1024, D] -> resident
            wo_sb = wopool.tile([128, NQH, D], BF16)
            wotr = wot.rearrange("(et p) d -> p et d", p=128)
            nc.sync.dma_start(wo_sb[:, :, 0:512], wotr[:, :, 0:512])
            nc.sync.dma_start(wo_sb[:, :, 512:D], wotr[:, :, 512:D])
            for dch in range(8):
                yw = y_dram[dch].rearrange("(t p c) -> t p c", p=128, c=512)
                for qg in range(4):
                    yps = [ps_y.tile([128, 512], F32, tag=f"yps{st}",
                                     name=f"yps{st}") for st in range(4)]
                    for et in range(NQH):
                        for st in range(4):
                            qt = qg * 4 + st
                            nc.tensor.matmul(
                                yps[st][:],
                                attn_sb[:, et, qt * 128:(qt + 1) * 128],
                                wo_sb[:, et, dch * 512:(dch + 1) * 512],
                                start=(et == 0), stop=(et == NQH - 1))
                    for st in range(4):
                        y = ypool.tile([128, 512], F32, tag="y_sb")
                        nc.vector.tensor_copy(y[:], yps[st][:])
                        nc.sync.dma_start(yw[qg * 4 + st], y[:])
                nc.gpsimd.collective_compute(
                    "ReduceScatter", mybir.AluOpType.add,
                    replica_groups=REPLICA_GROUPS,
                    ins=[y_dram[dch].opt()], outs=[rs_out[dch].opt()])
                nc.gpsimd.dma_start(
                    out[:, dch * 512:(dch + 1) * 512],
                    rs_out[dch].rearrange("(q c) -> q c", c=512))


# ======================= host side =======================

def _perm_idx(nheads):
    """Within each 128-dim head block: evens then odds."""
    idx = []
    for hh in range(nheads):
        base = hh * HD
        idx.extend(base + j for j in range(0, HD, 2))
        idx.extend(base + j for j in range(1, HD, 2))
    return np.array(idx)


def host_prep(x_norm, wq, wk, wv, wo, freqs_cos, freqs_sin, mask):
    """Build the 8 per-core input maps."""
    import ml_dtypes
    bf16 = ml_dtypes.bfloat16
    f32 = np.float32
    x_norm = np.ascontiguousarray(x_norm, dtype=f32)
    wq = np.asarray(wq, dtype=f32) * SCALE   # fold 1/sqrt(HD) into wq
    wk = np.asarray(wk, dtype=f32)
    wv = np.asarray(wv, dtype=f32)
    wo = np.asarray(wo, dtype=f32)
    fc = np.asarray(freqs_cos, dtype=f32)
    fs = np.asarray(freqs_sin, dtype=f32)

    pq = _perm_idx(H)
    pk = _perm_idx(KVH)
    wq_p = wq[pq, :]                     # [H*HD, D] permuted rows
    wk_p = wk[pk, :]

    cosT = fc.T                          # [64, S]
    sinT = fs.T
    cos_full = np.concatenate([cosT, cosT], axis=0).astype(bf16)   # [128, S]
    sin_full = np.concatenate([-sinT, sinT], axis=0).astype(bf16)

    # diagonal 0/1 mask, block-invariant: [k_rel 128, half, q 256]
    q_r = np.arange(256)
    k_r = np.arange(128)
    md = np.zeros((128, 2, 256), dtype=f32)
    md[:, 0, :] = (q_r[None, :] >= k_r[:, None])
    md[:, 1, :] = (q_r[None, :] >= (128 + k_r)[:, None])
    md = np.ascontiguousarray(md.reshape(128, 512)).astype(bf16)

    woT = wo.T                                       # [H*HD, D]

    xt_b = [np.ascontiguousarray(x_norm[b].T).astype(bf16) for b in range(B)]

    in_maps = []
    for c in range(8):
        b, hg = c // 4, c % 4
        # wq slice: heads [8hg, 8hg+8) -> [NQH, NDT, 128, 128]
        wq_c = wq_p[hg * NQH * HD:(hg + 1) * NQH * HD, :]   # [1024, 4096]
        wq_t = wq_c.T.reshape(NDT, 128, NQH, 128)            # [dt, p, et, e]
        wqt = np.ascontiguousarray(wq_t.transpose(2, 0, 1, 3)).astype(bf16)
        # wk/wv slice: kv-heads [2hg, 2hg+2) -> [NDT, 128, 256]
        wk_c = wk_p[hg * NKVH * HD:(hg + 1) * NKVH * HD, :]
        wkt = np.ascontiguousarray(
            wk_c.T.reshape(NDT, 128, NKVH * 128)).astype(bf16)
        wv_c = wv[hg * NKVH * HD:(hg + 1) * NKVH * HD, :]
        wvt = np.ascontiguousarray(
            wv_c.T.reshape(NDT, 128, NKVH * 128)).astype(bf16)
        # wo rows for this core's 8 heads
        wot = np.ascontiguousarray(
            woT[hg * NQH * HD:(hg + 1) * NQH * HD, :]).astype(bf16)

        in_maps.append({
            "xt": xt_b[b],
            "wqt": wqt, "wkt": wkt, "wvt": wvt, "wot": wot,
            "cos2": cos_full, "sin2": sin_full, "mdiag": md,
        })
    return in_maps


def assemble(results):
    """results: list of 8 dicts with 'out' [512, 4096] -> full [B, S, D]."""
    full = np.empty((B, S, D), dtype=np.float32)
    for c in range(8):
        b, hg = c // 4, c % 4
        full[b, hg * 512:(hg + 1) * 512] = results[c]["out"]
    return full


# ======================= public entry point =======================

_NC_CACHE = {}
last_exec_time_ns = None


def _get_nc():
    if "nc" not in _NC_CACHE:
        _NC_CACHE["nc"] = build()
    return _NC_CACHE["nc"]


def kernel(x_norm, wq, wk, wv, wo, freqs_cos, freqs_sin, mask, start_pos=0, **_):
    """GQA attention prefill on 8 TRN2 NeuronCores. Full inputs in, full output out."""
    import os
    global last_exec_time_ns
    from concourse.bass_utils import run_bass_kernel_spmd

    nc = _get_nc()
    in_maps = host_prep(x_norm, wq, wk, wv, wo, freqs_cos, freqs_sin, mask)
    trace = os.environ.get("BASS_KERNEL_TRACE", "0") == "1"
    res = run_bass_kernel_spmd(nc, in_maps, core_ids=list(range(8)), trace=trace)
    last_exec_time_ns = res.exec_time_ns
    return assemble(res.results)
